# revision 1
# baseline (speedup 1.0000x reference)
"""BitSwarmLinear Trainium2 kernel.

Computation (reference):
    swarm_sum = population.sum(axis=2)          # (out, in)
    w_eff     = sign(swarm_sum), sign(0) -> +1  # (out, in), +-1
    y         = einsum("bsi,oi->bso", x, w_eff) # (4, 4096, out)

Distribution (8 NeuronCores, tensor-parallel on out_features):
    - population sharded on out_features: each core gets its 256 rows,
      reduces + binarizes them and computes its 256 output columns.
    - x replicated to every core, staged pre-transposed/tiled as bf16 so the
      contraction dim lands on SBUF partitions with fully-contiguous DMA.
    - outputs gathered on the host along the feature dim.

Host staging (lossless / layout-only):
    - population is exactly +-1.0 -> int8, rearranged swarm-major
      [32, out_c, in]: cuts the dominant input stream 4x and lets the DMA
      engines' inline CCE ALU do the swarm reduction during transfer.
    - x -> bf16 x^T, tiled [tb, 128 ki, 16 ko, TB tok] so every DMA line is
      a 32KB contiguous run (line-rate HBM).
    - y comes back bf16 tile-major; host restores [b, s, out] f32.

Per-core device pipeline:
    1. Four parallel SWDGE accumulate chains (8 DMAs each, CCE int8 add)
       reduce the swarm axis while transferring; DVE merges 4 partials,
       binarizes via (s >= 0) * 2 - 1 (exact: sums are even ints, 0 -> +1).
    2. PE-transpose the sign matrix into W [in(part), out] bf16 (SBUF
       resident, 1 MB).
    3. Stream x^T tiles (4MB contiguous DMAs, deep prefetch); per 128-token
       block accumulate 16 K-tile matmuls into PSUM [128 tok, 256 out]
       (fp32), round to bf16, store on the scalar HWDGE ring.
"""

import os
import sys

import numpy as np

for _p in ("/root/.axon_site/_ro/trn_rl_repo", "/opt/trn_rl_repo"):
    if os.path.isdir(_p) and _p not in sys.path:
        sys.path.append(_p)

import ml_dtypes

# bass_utils' axon trace path imports antenv.axon_hooks, which this image
# lacks. Provide it (backed by the ctypes NTFF hook) so running with
# BASS_TRACE=1 works instead of crashing on the import.
try:
    import antenv.axon_hooks  # noqa: F401
except ImportError:
    try:
        import types as _types

        from trn_agent_boot.trn_boot import _ntff_profile_via_ctypes

        _hooks = _types.ModuleType("antenv.axon_hooks")
        _ntff_hook = _ntff_profile_via_ctypes("/opt/axon/libaxon_pjrt.so")
        _hooks.get_axon_ntff_profile_hook = lambda: _ntff_hook
        _hooks.set_axon_ntff_profile_hook = lambda h: None
        sys.modules["antenv.axon_hooks"] = _hooks
    except Exception:
        pass

import concourse.bass as bass  # noqa: F401  (AP helpers)
import concourse.mybir as mybir
import concourse.tile as tile
from concourse import bacc
from concourse.bass_utils import run_bass_kernel_spmd
from concourse.masks import make_identity

P = 128
IN_F = 2048
SWARM = 32
OUT_F = 2048
N_CORES = 8
OUT_C = OUT_F // N_CORES  # 256 out features per core
TOKENS = 4 * 4096

F32 = mybir.dt.float32
BF16 = mybir.dt.bfloat16
U8 = mybir.dt.uint8
I16 = mybir.dt.int16

# token-block per x^T DMA / output store
TB = 1024
# x^T prefetch depth (SBUF: 32KB/partition each at TB=1024)
XT_BUFS = 4
# swarm-slice staging depth for the reduction
POP_BUFS = 4


def build_nc(tokens: int = TOKENS, out_c: int = OUT_C, in_f: int = IN_F,
             reps: int = 1):
    """Build the per-core Bass program (same program on all 8 cores).

    reps>1 repeats the whole pipeline back-to-back (timing harness only)."""
    ko_tiles = in_f // P          # 16 K-tiles
    oc_groups = out_c // P        # 2 groups of 128 out rows
    tb_count = tokens // TB
    m_per_tb = TB // P

    nc = bacc.Bacc(
        "TRN2",
        target_bir_lowering=False,
        debug=False,
        enable_asserts=False,
        num_devices=N_CORES,
    )

    xT = nc.dram_tensor("xT", [tb_count, P, ko_tiles, TB], BF16,
                        kind="ExternalInput")
    pop = nc.dram_tensor("pop", [SWARM, out_c, in_f], U8,
                         kind="ExternalInput")
    y = nc.dram_tensor("y", [tb_count, P, m_per_tb, out_c], BF16,
                       kind="ExternalOutput")

    xr = xT.ap()                                              # [tb,128,ko,TB]
    pr = pop.ap().rearrange("s (g p) i -> s p g i", p=P)      # [32,128,oc,in]
    yr = y.ap()                                               # [tb,128,m,oc*P]

    with tile.TileContext(nc) as tc:
        with (
            tc.tile_pool(name="const", bufs=1) as const_pool,
            tc.tile_pool(name="pops", bufs=POP_BUFS) as pop_pool,
            tc.tile_pool(name="acc", bufs=1) as acc_pool,
            tc.tile_pool(name="sgn", bufs=oc_groups) as sgn_pool,
            tc.tile_pool(name="wsb", bufs=1) as w_pool,
            tc.tile_pool(name="xt", bufs=XT_BUFS) as x_pool,
            tc.tile_pool(name="ystage", bufs=2) as y_pool,
            tc.tile_pool(name="psum_t", bufs=2, space="PSUM") as psum_t_pool,
            tc.tile_pool(name="psum_y", bufs=4, space="PSUM") as psum_y_pool,
        ):
            ident = const_pool.tile([P, P], F32)
            make_identity(nc, ident[:])

            for _rep in range(reps):
                _emit_body(
                    nc, ident, w_pool, pop_pool, acc_pool, sgn_pool, x_pool,
                    y_pool, psum_t_pool, psum_y_pool, pr, xr, yr,
                    oc_groups, ko_tiles, tb_count, m_per_tb, out_c, in_f,
                )

    nc.compile()  # bacc register allocation / DCE — required before codegen
    return nc


def _emit_body(nc, ident, w_pool, pop_pool, acc_pool, sgn_pool, x_pool,
               y_pool, psum_t_pool, psum_y_pool, pr, xr, yr,
               oc_groups, ko_tiles, tb_count, m_per_tb, out_c, in_f):
    # W in [in(part), ko, out] bf16 — matmul rhs tiles, SBUF-resident
    w_sb = w_pool.tile([P, ko_tiles, out_c], BF16, tag="wsb")

    # ---- Stage 1: swarm reduction as packed-byte adds.
    # pop is staged {0,1} uint8; 2 bytes are summed per int16 lane — no
    # carries cross byte lanes (every lane stays in [0, 32]), and int16
    # values <= 8224 survive the DVE's fp32 ALU exactly.
    acc = acc_pool.tile([P, oc_groups, in_f // 2], I16, tag="acc")
    for s in range(SWARM):
        pt = pop_pool.tile([P, oc_groups, in_f], U8, tag="pops")
        eng = nc.sync if s % 2 == 0 else nc.scalar
        eng.dma_start(pt[:], pr[s])
        if s == 0:
            nc.vector.tensor_copy(out=acc[:], in_=pt[:].bitcast(I16))
        else:
            nc.vector.tensor_add(acc[:], acc[:], pt[:].bitcast(I16))

    # ---- Stage 2: binarize + PE-transpose into W [in, out] bf16
    acc_u8 = acc[:].bitcast(U8)  # [128, oc, in] counts in [0, 32]
    for oc in range(oc_groups):
        sgn = sgn_pool.tile([P, in_f], F32, tag="sgn", name=f"sgn{oc}")
        # count >= 16  <=>  swarm_sum >= 0; w = (count >= 16) * 2 - 1
        nc.vector.tensor_scalar(
            out=sgn[:], in0=acc_u8[:, oc, :], scalar1=16, scalar2=2.0,
            op0=mybir.AluOpType.is_ge, op1=mybir.AluOpType.mult,
        )
        nc.vector.tensor_scalar(
            out=sgn[:], in0=sgn[:], scalar1=1.0, scalar2=None,
            op0=mybir.AluOpType.subtract,
        )
        for k in range(ko_tiles):
            pt_ps = psum_t_pool.tile([P, P], F32, tag="tps")
            nc.tensor.transpose(
                pt_ps[:], sgn[:, k * P : (k + 1) * P], ident[:]
            )
            nc.vector.tensor_copy(
                out=w_sb[:, k, oc * P : (oc + 1) * P], in_=pt_ps[:]
            )

    # ---- Stage 3: stream x^T, matmul, store y (bf16)
    for tb in range(tb_count):
        xt = x_pool.tile([P, ko_tiles, TB], BF16, tag="xt")
        nc.sync.dma_start(xt[:], xr[tb])
        ystage = y_pool.tile([P, m_per_tb, out_c], BF16, tag="ys")
        for m in range(m_per_tb):
            ps = psum_y_pool.tile([P, out_c], F32, tag="yps")
            for k in range(ko_tiles):
                nc.tensor.matmul(
                    ps[:],
                    xt[:, k, m * P : (m + 1) * P],
                    w_sb[:, k, :],
                    start=(k == 0),
                    stop=(k == ko_tiles - 1),
                )
            nc.vector.tensor_copy(out=ystage[:, m, :], in_=ps[:])
        # stores ride the ACT HWDGE ring; loads own the SP ring
        nc.scalar.dma_start(yr[tb], ystage[:])


_NC_CACHE: dict = {}


def _get_nc(tokens=TOKENS, out_c=OUT_C, in_f=IN_F):
    key = (tokens, out_c, in_f)
    if key not in _NC_CACHE:
        _NC_CACHE[key] = build_nc(*key)
    return _NC_CACHE[key]


def stage_x(x: np.ndarray, tokens: int, in_f: int):
    """x [b, s, in] f32 -> tiled bf16 [tb, 128 ki, ko, TB] of x^T."""
    xb = np.ascontiguousarray(
        x.reshape(tokens, in_f).T
    ).astype(ml_dtypes.bfloat16)  # [in, tokens]
    ko = in_f // P
    tb = tokens // TB
    # (ko ki) (tb t) -> tb ki ko t
    return np.ascontiguousarray(
        xb.reshape(ko, P, tb, TB).transpose(2, 1, 0, 3)
    )


def stage_pop_slice(pop_c: np.ndarray):
    """pop slice [out_c, in, 32] (+-1.0 f32) -> swarm-major {0,1} uint8
    [32, out_c, in]. Lossless recode: -1 -> 0, +1 -> 1."""
    return np.ascontiguousarray(
        (pop_c > 0).astype(np.uint8).transpose(2, 0, 1)
    )


def unstage_y(y_dev: np.ndarray, tokens: int, out_c: int):
    """y [tb, 128 p, m, out_c] bf16 -> [tokens, out_c] f32
    (token = tb*TB + m*128 + p)."""
    return (
        y_dev.astype(np.float32)
        .transpose(0, 2, 1, 3)
        .reshape(tokens, out_c)
    )


def prep_inputs(x: np.ndarray, population: np.ndarray):
    tokens = x.shape[0] * x.shape[1]
    in_f = x.shape[2]
    xT = stage_x(x, tokens, in_f)
    out_c = population.shape[0] // N_CORES
    in_maps = []
    for c in range(N_CORES):
        pop_c = stage_pop_slice(population[c * out_c : (c + 1) * out_c])
        in_maps.append({"xT": xT, "pop": pop_c})
    return in_maps, tokens, out_c, in_f


def kernel(x: np.ndarray, population: np.ndarray):
    in_maps, tokens, out_c, in_f = prep_inputs(x, population)
    nc = _get_nc(tokens, out_c, in_f)
    res = run_bass_kernel_spmd(nc, in_maps, core_ids=list(range(N_CORES)))
    y_full = np.concatenate(
        [unstage_y(r["y"], tokens, out_c) for r in res.results], axis=1
    )
    return y_full.reshape(x.shape[0], x.shape[1], population.shape[0])



# revision 3
# speedup vs baseline: 1.2033x; 1.2033x over previous
"""BitSwarmLinear Trainium2 kernel.

Computation (reference):
    swarm_sum = population.sum(axis=2)          # (out, in)
    w_eff     = sign(swarm_sum), sign(0) -> +1  # (out, in), +-1
    y         = einsum("bsi,oi->bso", x, w_eff) # (4, 4096, out)

Distribution (8 NeuronCores, tensor-parallel on out_features):
    - population sharded on out_features: each core gets its 256 rows,
      reduces + binarizes them and computes its 256 output columns.
    - x replicated to every core, staged pre-transposed/tiled as bf16 so the
      contraction dim lands on SBUF partitions with fully-contiguous DMA.
    - outputs gathered on the host along the feature dim.

Host staging (lossless / layout-only):
    - population bits nibble-packed, two swarm planes per byte
      (plane j<15: s_j | s_{15+j}<<4; plane 15: s_30 | s_31<<4), laid out
      IN-major [chunk, in%128, plane, in//128, out] so the reduction output
      lands directly in matmul-rhs orientation (no PE transpose) and each
      chunk DMA is 128 fat 16KB descriptors. 8.4 MB/core (2x less than u8).
    - x -> bf16 x^T, tiled [tb, 128 ki, 16 ko, TB tok]: contiguous lines.
    - y returns bf16 tile-major; host restores [b, s, out] f32.

Per-core device pipeline:
    1. Four pop chunk DMAs (2.1 MB each, split across the SP + ACT HWDGE
       rings, nothing else competing). DVE accumulates packed planes as
       uint16 lanes (exact: byte sums <= 255, lanes <= 65535 < 2^24 survive
       the fp32 ALU; no cross-byte carry possible).
    2. DVE nibble-unpack (shift/mask/add) -> per-byte counts in [0,32];
       binarize via (count >= 16)*2-1 == sign(swarm_sum) with sign(0)->+1,
       written straight into W [in(part), ko, out] bf16 (matmul rhs).
    3. Stream x^T in ko-quarter slices (1 MB, 8KB/partition runs), gated
       behind the binarize for the first XT_BUFS tiles so the x stream can
       never starve the pop reduction. Per 128-token block: 16 K-tile
       matmuls accumulate PSUM [128 tok, 256 out] fp32 (16 PSUM tiles =
       all 8 banks, drains fully off the critical path); DVE casts to
       bf16, stores ride the ACT ring.
"""

import os
import sys

import numpy as np

for _p in ("/root/.axon_site/_ro/trn_rl_repo", "/opt/trn_rl_repo"):
    if os.path.isdir(_p) and _p not in sys.path:
        sys.path.append(_p)

import ml_dtypes

# bass_utils' axon trace path imports antenv.axon_hooks, which this image
# lacks. Provide it (backed by the ctypes NTFF hook) so running with
# BASS_TRACE=1 works instead of crashing on the import.
try:
    import antenv.axon_hooks  # noqa: F401
except ImportError:
    try:
        import types as _types

        from trn_agent_boot.trn_boot import _ntff_profile_via_ctypes

        _hooks = _types.ModuleType("antenv.axon_hooks")
        _ntff_hook = _ntff_profile_via_ctypes("/opt/axon/libaxon_pjrt.so")
        _hooks.get_axon_ntff_profile_hook = lambda: _ntff_hook
        _hooks.set_axon_ntff_profile_hook = lambda h: None
        sys.modules["antenv.axon_hooks"] = _hooks
    except Exception:
        pass

import concourse.bass as bass  # noqa: F401  (AP helpers)
import concourse.mybir as mybir
import concourse.tile as tile
from concourse import bacc
from concourse.bass_utils import run_bass_kernel_spmd

P = 128
IN_F = 2048
SWARM = 32
OUT_F = 2048
N_CORES = 8
OUT_C = OUT_F // N_CORES  # 256 out features per core
TOKENS = 4 * 4096

F32 = mybir.dt.float32
BF16 = mybir.dt.bfloat16
U8 = mybir.dt.uint8
U16 = mybir.dt.uint16

# token-block per x^T tile / output store
TB = 1024
# x^T prefetch depth (SBUF: 32KB/partition each at TB=1024)
XT_BUFS = 4
# packed pop planes per chunk DMA (4 chunks x 4 planes)
PK_CHUNKS = 4
PK_PER_CHUNK = 4


def build_nc(tokens: int = TOKENS, out_c: int = OUT_C, in_f: int = IN_F,
             reps: int = 1):
    """Build the per-core Bass program (same program on all 8 cores).

    reps>1 repeats the whole pipeline back-to-back (timing harness only)."""
    ko_tiles = in_f // P          # 16 K-tiles
    tb_count = tokens // TB
    m_per_tb = TB // P

    nc = bacc.Bacc(
        "TRN2",
        target_bir_lowering=False,
        debug=False,
        enable_asserts=False,
        num_devices=N_CORES,
    )

    xT = nc.dram_tensor("xT", [tb_count, P, ko_tiles, TB], BF16,
                        kind="ExternalInput")
    pop = nc.dram_tensor("pop", [PK_CHUNKS, P, PK_PER_CHUNK, ko_tiles, out_c],
                         U8, kind="ExternalInput")
    y = nc.dram_tensor("y", [tb_count, P, m_per_tb, out_c], BF16,
                       kind="ExternalOutput")

    xr = xT.ap()                                              # [tb,128,ko,TB]
    pr = pop.ap()                                             # [4,128,4,ko,oc]
    yr = y.ap()                                               # [tb,128,m,oc]

    with tile.TileContext(nc) as tc:
        with (
            tc.tile_pool(name="pops", bufs=2) as pop_pool,
            tc.tile_pool(name="red", bufs=1) as red_pool,
            tc.tile_pool(name="wsb", bufs=1) as w_pool,
            tc.tile_pool(name="xt", bufs=XT_BUFS) as x_pool,
            tc.tile_pool(name="ystage", bufs=2) as y_pool,
            tc.tile_pool(name="psum_y", bufs=8, space="PSUM") as psum_pool,
        ):
            for _rep in range(reps):
                _emit_body(
                    nc, pop_pool, red_pool, w_pool, x_pool, y_pool,
                    psum_pool, pr, xr, yr, ko_tiles, tb_count, m_per_tb,
                    out_c, in_f,
                )

    nc.compile()  # bacc register allocation / DCE — required before codegen
    return nc


def _emit_body(nc, pop_pool, red_pool, w_pool, x_pool, y_pool, psum_pool,
               pr, xr, yr, ko_tiles, tb_count, m_per_tb, out_c, in_f):
    lanes = ko_tiles * out_c // 2  # uint16 lanes per partition

    # W in [in(part), ko, out] bf16 — matmul rhs tiles, SBUF-resident
    w_sb = w_pool.tile([P, ko_tiles, out_c], BF16, tag="wsb")

    # ---- Stage 1: swarm reduction over nibble-packed planes.
    # Chunks land on alternating HWDGE rings; DVE adds uint16 lane views
    # (exact in the fp32 ALU, no cross-byte carries by construction).
    acc = red_pool.tile([P, lanes], U16, tag="acc")
    t1 = red_pool.tile([P, lanes], U16, tag="t1")
    t2 = red_pool.tile([P, lanes], U16, tag="t2")
    t3 = red_pool.tile([P, lanes], U16, tag="t3")

    pk_tiles = []
    for c in range(PK_CHUNKS):
        pt = pop_pool.tile([P, PK_PER_CHUNK, ko_tiles, out_c], U8, tag="pops")
        eng = nc.sync if c % 2 == 0 else nc.scalar
        eng.dma_start(pt[:], pr[c])
        pk_tiles.append(pt)

    n_planes_acc = PK_CHUNKS * PK_PER_CHUNK - 1  # 15 accumulated; last is pB
    idx = 0
    pB = None
    for c in range(PK_CHUNKS):
        pv = pk_tiles[c][:].bitcast(U16)  # [128, 4, lanes... ] halves last dim
        for j in range(PK_PER_CHUNK):
            if idx == n_planes_acc:
                pB = pv[:, j]
            elif idx == 0:
                nc.vector.tensor_copy(out=acc[:], in_=pv[:, j])
            else:
                nc.vector.tensor_add(acc[:], acc[:], pv[:, j])
            idx += 1

    # ---- Stage 2: nibble-unpack + binarize directly into W (bf16).
    # count = (acc>>4 & 0x0F0F) + (acc & 0x0F0F) + (pB>>4 & 0x0F0F)
    #         + (pB & 0x0F0F); per-byte counts in [0, 32].
    nc.vector.tensor_scalar(out=t1[:], in0=acc[:], scalar1=4, scalar2=0x0F0F,
                            op0=mybir.AluOpType.logical_shift_right,
                            op1=mybir.AluOpType.bitwise_and)
    nc.vector.tensor_scalar(out=t2[:], in0=acc[:], scalar1=0x0F0F,
                            scalar2=None, op0=mybir.AluOpType.bitwise_and)
    nc.vector.tensor_add(t1[:], t1[:], t2[:])
    nc.vector.tensor_scalar(out=t2[:], in0=pB, scalar1=4, scalar2=0x0F0F,
                            op0=mybir.AluOpType.logical_shift_right,
                            op1=mybir.AluOpType.bitwise_and)
    nc.vector.tensor_scalar(out=t3[:], in0=pB, scalar1=0x0F0F,
                            scalar2=None, op0=mybir.AluOpType.bitwise_and)
    nc.vector.tensor_add(t2[:], t2[:], t3[:])
    nc.vector.tensor_add(t1[:], t1[:], t2[:])

    cnt_u8 = t1[:].bitcast(U8).rearrange("p (k o) -> p k o", k=ko_tiles)
    # count >= 16  <=>  swarm_sum >= 0; w = (count >= 16)*2 - 1 (0 -> +1)
    nc.vector.tensor_scalar(out=w_sb[:], in0=cnt_u8, scalar1=16, scalar2=2.0,
                            op0=mybir.AluOpType.is_ge,
                            op1=mybir.AluOpType.mult)
    nc.vector.tensor_scalar(out=w_sb[:], in0=w_sb[:], scalar1=1.0,
                            scalar2=None, op0=mybir.AluOpType.subtract)

    # ---- Stage 3: stream x^T in ko-quarter slices, matmul, store y (bf16)
    ko_q = ko_tiles // 4
    for tb in range(tb_count):
        xt = x_pool.tile([P, ko_tiles, TB], BF16, tag="xt")
        if tb < XT_BUFS:
            # Order the x stream behind the binarize (WAW on a sliver of
            # the tile) so it cannot race the pop reduction for DMA
            # bandwidth. Later tiles are gated by buffer reuse anyway.
            nc.vector.tensor_copy(out=xt[0:1, :, 0:1], in_=w_sb[0:1, :, 0:1])
        for q in range(4):
            eng = nc.sync if (tb * 4 + q) % 2 == 0 else nc.scalar
            eng.dma_start(xt[:, q * ko_q:(q + 1) * ko_q, :],
                          xr[tb, :, q * ko_q:(q + 1) * ko_q, :])
        ystage = y_pool.tile([P, m_per_tb, out_c], BF16, tag="ys")
        for m in range(m_per_tb):
            ps = psum_pool.tile([P, out_c], F32, tag="yps")
            for k in range(ko_tiles):
                nc.tensor.matmul(
                    ps[:],
                    xt[:, k, m * P:(m + 1) * P],
                    w_sb[:, k, :],
                    start=(k == 0),
                    stop=(k == ko_tiles - 1),
                )
            nc.vector.tensor_copy(out=ystage[:, m, :], in_=ps[:])
        nc.scalar.dma_start(yr[tb], ystage[:])


_NC_CACHE: dict = {}


def _get_nc(tokens=TOKENS, out_c=OUT_C, in_f=IN_F):
    key = (tokens, out_c, in_f)
    if key not in _NC_CACHE:
        _NC_CACHE[key] = build_nc(*key)
    return _NC_CACHE[key]


def stage_x(x: np.ndarray, tokens: int, in_f: int):
    """x [b, s, in] f32 -> tiled bf16 [tb, 128 ki, ko, TB] of x^T."""
    xb = np.ascontiguousarray(
        x.reshape(tokens, in_f).T
    ).astype(ml_dtypes.bfloat16)  # [in, tokens]
    ko = in_f // P
    tb = tokens // TB
    # (ko ki) (tb t) -> tb ki ko t
    return np.ascontiguousarray(
        xb.reshape(ko, P, tb, TB).transpose(2, 1, 0, 3)
    )


def stage_pop_slice(pop_c: np.ndarray):
    """pop slice [out_c, in, 32] (+-1.0 f32) -> nibble-packed swarm planes
    [4 chunk, 128 p, 4 plane, ko, out_c] u8. Lossless bit-repack:
    plane j<15 holds s_j | s_{15+j}<<4; plane 15 holds s_30 | s_31<<4."""
    out_c, in_f, _ = pop_c.shape
    ko = in_f // P
    bits = (pop_c > 0).astype(np.uint8).transpose(2, 1, 0)  # [32, in, out_c]
    planes = np.empty((16, in_f, out_c), np.uint8)
    planes[:15] = bits[:15] | (bits[15:30] << 4)
    planes[15] = bits[30] | (bits[31] << 4)
    # [16 j, (ko p) in, oc] -> [4 c, 128 p, 4 jj, ko, oc]
    arr = planes.reshape(4, 4, ko, P, out_c).transpose(0, 3, 1, 2, 4)
    return np.ascontiguousarray(arr)


def unstage_y(y_dev: np.ndarray, tokens: int, out_c: int):
    """y [tb, 128 p, m, out_c] bf16 -> [tokens, out_c] f32
    (token = tb*TB + m*128 + p)."""
    return (
        y_dev.astype(np.float32)
        .transpose(0, 2, 1, 3)
        .reshape(tokens, out_c)
    )


def prep_inputs(x: np.ndarray, population: np.ndarray):
    tokens = x.shape[0] * x.shape[1]
    in_f = x.shape[2]
    xT = stage_x(x, tokens, in_f)
    out_c = population.shape[0] // N_CORES
    in_maps = []
    for c in range(N_CORES):
        pop_c = stage_pop_slice(population[c * out_c:(c + 1) * out_c])
        in_maps.append({"xT": xT, "pop": pop_c})
    return in_maps, tokens, out_c, in_f


def kernel(x: np.ndarray, population: np.ndarray):
    in_maps, tokens, out_c, in_f = prep_inputs(x, population)
    nc = _get_nc(tokens, out_c, in_f)
    res = run_bass_kernel_spmd(nc, in_maps, core_ids=list(range(N_CORES)))
    y_full = np.concatenate(
        [unstage_y(r["y"], tokens, out_c) for r in res.results], axis=1
    )
    return y_full.reshape(x.shape[0], x.shape[1], population.shape[0])
